# revision 1
# baseline (speedup 1.0000x reference)
"""Trainium2 Bass kernel for nn_DirectContractedVoxGO_Sto.

Data-parallel over rays: 8 cores x 512 rays. Grids repacked host-side to
[x,y,z,28] bf16 (14 channels at (x,y,z) + 14 at (x,y+1,z)) so each trilinear
(x-corner) needs ONE contiguous 56-value window (covers the (y,z) 2x2 quad);
two indirect-DMA windows per sample point. On-chip: separable trilinear blend
(DVE), transmittance via tensor_tensor_scan cumprod (no log/exp cumsum),
tiny MLP on PE with per-sample transposes, stochastic RGB via tanh-sigmoid,
and a fused (s,k) reduction.
"""
import numpy as np
import ml_dtypes

import concourse.bass as bass
import concourse.bacc as bacc
import concourse.mybir as mybir
import concourse.tile as tile
from concourse.bass_utils import run_bass_kernel_spmd

bfm = ml_dtypes.bfloat16
F32 = mybir.dt.float32
BF16 = mybir.dt.bfloat16
I32 = mybir.dt.int32
AF = mybir.ActivationFunctionType
OP = mybir.AluOpType

# problem constants (hardcoded; kernel.py must be self-contained)
G = 160
S = 256
K = 12
RPC = 512            # rays per core
NC = 8
XYZ_MIN = -1.2
XYZ_MAX = 1.2
ACT_SHIFT = float(np.log(1.0 / (1.0 - 1e-4) - 1.0))
STEPDIST = 0.5 * (XYZ_MAX - XYZ_MIN) / G
NEAR = 0.1
INTERVAL = 0.5
NENT = G * G * G
USCL = (G - 1) / (XYZ_MAX - XYZ_MIN)
SBLK = 32            # samples per blend block
MBLK = 4             # samples per MLP block


def build_program():
    nc = bacc.Bacc("TRN2", target_bir_lowering=False, debug=False, num_devices=NC)
    for val in {float(np.pi / 2), -0.5, -1.0, ACT_SHIFT, 0.5, -float(INTERVAL)}:
        t = nc.alloc_sbuf_tensor(f"constx-{val}", [128, 1], F32)
        nc.gpsimd.memset(t.ap(), val)
        nc.const_aps.aps[(F32, val)] = t.ap()
    nc.all_engine_barrier()
    gridp = nc.dram_tensor("gridp", [NENT, 28], BF16, kind="ExternalInput")
    rays_o = nc.dram_tensor("rays_o", [RPC, 3], F32, kind="ExternalInput")
    rays_d = nc.dram_tensor("rays_d", [RPC, 3], F32, kind="ExternalInput")
    t_rep = nc.dram_tensor("t_rep", [128, S], F32, kind="ExternalInput")
    freq_rep = nc.dram_tensor("freq_rep", [128, 12], F32, kind="ExternalInput")
    eps_rep = nc.dram_tensor("eps_rep", [128, K], F32, kind="ExternalInput")
    epsr_rep = nc.dram_tensor("epsr_rep", [128, K * 3], F32, kind="ExternalInput")
    w0T = nc.dram_tensor("w0T", [39, 128], F32, kind="ExternalInput")
    w1T = nc.dram_tensor("w1T", [128, 128], F32, kind="ExternalInput")
    w2T = nc.dram_tensor("w2T", [128, 6], F32, kind="ExternalInput")
    b0c = nc.dram_tensor("b0c", [128, 1], F32, kind="ExternalInput")
    b1c = nc.dram_tensor("b1c", [128, 1], F32, kind="ExternalInput")
    b2c = nc.dram_tensor("b2c", [6, 1], F32, kind="ExternalInput")
    identd = nc.dram_tensor("identd", [128, 128], F32, kind="ExternalInput")
    vembd = nc.dram_tensor("vembd", [RPC, 27], F32, kind="ExternalInput")
    out = nc.dram_tensor("out", [RPC, 3], F32, kind="ExternalOutput")

    with tile.TileContext(nc) as tc:
        with tc.tile_pool(name="const", bufs=1) as cp, \
             tc.tile_pool(name="big", bufs=1) as bp, \
             tc.tile_pool(name="wk", bufs=2) as wk, \
             tc.tile_pool(name="win", bufs=2) as winp, \
             tc.tile_pool(name="mlp", bufs=1) as mp, \
             tc.tile_pool(name="ps", bufs=2, space="PSUM") as psp, \
             tc.tile_pool(name="ps1", bufs=1, space="PSUM") as ps1, \
             tc.tile_pool(name="ps2", bufs=1, space="PSUM") as ps2:

            tt = cp.tile([128, S], F32, tag="t_rep")
            nc.sync.dma_start(out=tt[:], in_=t_rep[:])
            fq = cp.tile([128, 12], F32, tag="freq")
            nc.sync.dma_start(out=fq[:], in_=freq_rep[:])
            ep = cp.tile([128, K], F32, tag="eps")
            nc.sync.dma_start(out=ep[:], in_=eps_rep[:])
            epr = cp.tile([128, K * 3], F32, tag="epsr")
            nc.sync.dma_start(out=epr[:], in_=epsr_rep[:])
            w0t = cp.tile([39, 128], F32, tag="w0")
            nc.sync.dma_start(out=w0t[:], in_=w0T[:])
            w1t = cp.tile([128, 128], F32, tag="w1")
            nc.sync.dma_start(out=w1t[:], in_=w1T[:])
            w2t = cp.tile([128, 6], F32, tag="w2")
            nc.sync.dma_start(out=w2t[:], in_=w2T[:])
            b0t = cp.tile([128, 1], F32, tag="b0")
            nc.sync.dma_start(out=b0t[:], in_=b0c[:])
            b1t = cp.tile([128, 1], F32, tag="b1")
            nc.sync.dma_start(out=b1t[:], in_=b1c[:])
            b2t = cp.tile([6, 1], F32, tag="b2")
            nc.sync.dma_start(out=b2t[:], in_=b2c[:])
            idt = cp.tile([128, 128], F32, tag="ident")
            nc.sync.dma_start(out=idt[:], in_=identd[:])

            for rt in range(RPC // 128):
                r0 = rt * 128
                # ---- stage 0: ray setup ----
                ro = wk.tile([128, 3], F32, tag="ro")
                nc.sync.dma_start(out=ro[:], in_=rays_o[r0:r0 + 128, :])
                rd = wk.tile([128, 3], F32, tag="rd")
                nc.sync.dma_start(out=rd[:], in_=rays_d[r0:r0 + 128, :])
                rdsq = wk.tile([128, 3], F32, tag="rdsq")
                nc.scalar.activation(rdsq[:], rd[:], AF.Square)
                n2 = wk.tile([128, 1], F32, tag="n2")
                nc.vector.tensor_reduce(n2[:], rdsq[:], mybir.AxisListType.X, OP.add)
                nc.scalar.activation(n2[:], n2[:], AF.Ln)
                nc.scalar.activation(n2[:], n2[:], AF.Exp, scale=-0.5)  # rsqrt
                vd = wk.tile([128, 3], F32, tag="vd")
                nc.vector.tensor_scalar(vd[:], rd[:], n2[:], None, OP.mult)
                # vemb [128, 27] = [vd, sin, cos]  (host-computed)
                vemb = wk.tile([128, 27], F32, tag="vemb")
                nc.sync.dma_start(out=vemb[:], in_=vembd[r0:r0 + 128, :])
                # ---- stage A: pts / contraction / indices [128, S] ----
                pc = []
                for c in range(3):
                    pct = wk.tile([128, S], F32, tag=f"p{c}", name=f"p{c}")
                    pc.append(pct)
                for c in range(3):
                    nc.vector.tensor_scalar(pc[c][:], tt[:], vd[:, c:c + 1],
                                            ro[:, c:c + 1], OP.mult, OP.add)
                m = wk.tile([128, S], F32, tag="m")
                ab = wk.tile([128, S], F32, tag="ab")
                nc.scalar.activation(m[:], pc[0][:], AF.Abs)
                nc.scalar.activation(ab[:], pc[1][:], AF.Abs)
                nc.vector.tensor_tensor(m[:], m[:], ab[:], OP.max)
                nc.scalar.activation(ab[:], pc[2][:], AF.Abs)
                nc.vector.tensor_tensor(m[:], m[:], ab[:], OP.max)
                nc.vector.tensor_scalar(m[:], m[:], 1.0, None, OP.max)
                nc.scalar.activation(m[:], m[:], AF.Ln)
                rr = wk.tile([128, S], F32, tag="rr")
                nc.scalar.activation(rr[:], m[:], AF.Exp, scale=-1.0)
                rsq = wk.tile([128, S], F32, tag="rsq")
                nc.scalar.activation(rsq[:], rr[:], AF.Square)
                ff = wk.tile([128, S], F32, tag="ff")
                nc.vector.tensor_scalar(ff[:], rr[:], 1.2, None, OP.mult)
                nc.vector.tensor_scalar(rsq[:], rsq[:], 0.2, None, OP.mult)
                nc.vector.tensor_tensor(ff[:], ff[:], rsq[:], OP.subtract)
                frs = []
                ent = wk.tile([128, S], F32, tag="ent")
                for c in range(3):
                    u = wk.tile([128, S], F32, tag=f"u{c}")
                    nc.vector.tensor_tensor(u[:], pc[c][:], ff[:], OP.mult)
                    nc.vector.tensor_scalar(u[:], u[:], USCL, -XYZ_MIN * USCL,
                                            OP.mult, OP.add)
                    nc.vector.tensor_scalar(u[:], u[:], 0.0, None, OP.max)
                    fm = wk.tile([128, S], I32, tag=f"fm{c}", name="fm")
                    nc.vector.tensor_copy(fm[:], u[:])
                    i0 = wk.tile([128, S], F32, tag=f"i0{c}", name="i0")
                    nc.vector.tensor_copy(i0[:], fm[:])
                    gtm = wk.tile([128, S], F32, tag=f"gtm{c}", name="gtm")
                    nc.vector.tensor_tensor(gtm[:], i0[:], u[:], OP.is_gt)
                    nc.vector.tensor_tensor(i0[:], i0[:], gtm[:], OP.subtract)
                    nc.vector.tensor_scalar(i0[:], i0[:], float(G - 2), None, OP.min)
                    fr = wk.tile([128, S], F32, tag=f"fr{c}")
                    nc.vector.tensor_tensor(fr[:], u[:], i0[:], OP.subtract)
                    frs.append(fr)
                    if c == 0:
                        nc.vector.tensor_scalar(ent[:], i0[:], float(G), None, OP.mult)
                    elif c == 1:
                        nc.vector.tensor_tensor(ent[:], ent[:], i0[:], OP.add)
                        nc.vector.tensor_scalar(ent[:], ent[:], float(G), None, OP.mult)
                    else:
                        nc.vector.tensor_tensor(ent[:], ent[:], i0[:], OP.add)
                gxs = []
                for c in range(3):
                    gg = wk.tile([128, S], F32, tag=f"g{c}")
                    nc.vector.tensor_scalar(gg[:], frs[c][:], -1.0, 1.0, OP.mult, OP.add)
                    gxs.append(gg)
                fx, fy, fz = frs
                gx, gy, gz = gxs
                e0i = wk.tile([128, S], I32, tag="e0i")
                nc.vector.tensor_copy(e0i[:], ent[:])
                e1i = wk.tile([128, S], I32, tag="e1i")
                nc.vector.tensor_scalar(ent[:], ent[:], float(G * G), None, OP.add)
                nc.vector.tensor_copy(e1i[:], ent[:])

                # big per-rtile tiles
                k0b = bp.tile([128, S, 12], BF16, tag="k0b")
                denstd = bp.tile([128, S, 2], F32, tag="denstd")
                m6 = bp.tile([128, S, 6], F32, tag="m6")

                # ---- stages B+C: gather + blend, per 32-sample block ----
                for blk in range(S // SBLK):
                    sb = blk * SBLK
                    win = winp.tile([128, SBLK, 112], BF16, tag="win")
                    for si in range(SBLK):
                        s = sb + si
                        nc.gpsimd.indirect_dma_start(
                            out=win[:, si, 0:56], out_offset=None, in_=gridp[:],
                            in_offset=bass.IndirectOffsetOnAxis(
                                ap=e0i[:, s:s + 1], axis=0))
                        nc.gpsimd.indirect_dma_start(
                            out=win[:, si, 56:112], out_offset=None, in_=gridp[:],
                            in_offset=bass.IndirectOffsetOnAxis(
                                ap=e1i[:, s:s + 1], axis=0))
                    # blend weights as [128, SBLK] slices, broadcast over chans
                    def wbc(t, n):
                        return t[:, sb:sb + SBLK][:, :, None].broadcast_to(
                            [128, SBLK, n])
                    # z-blend: win [x2, z2, y2c14=28] -> zb [128, SBLK, 2, 28]
                    zb = wk.tile([128, SBLK, 2, 28], BF16, tag="zb")
                    zt = wk.tile([128, SBLK, 2, 28], BF16, tag="zt")
                    w4 = win[:].rearrange("p s (x z q) -> p s x z q", x=2, z=2)
                    nc.vector.tensor_tensor(
                        zb[:], w4[:, :, :, 0, :],
                        wbc(gz, 28)[:, :, None, :].broadcast_to([128, SBLK, 2, 28]),
                        OP.mult)
                    nc.vector.tensor_tensor(
                        zt[:], w4[:, :, :, 1, :],
                        wbc(fz, 28)[:, :, None, :].broadcast_to([128, SBLK, 2, 28]),
                        OP.mult)
                    nc.vector.tensor_tensor(zb[:], zb[:], zt[:], OP.add)
                    # y-blend: zb [x2, y2, 14] -> yb [128, SBLK, 2, 14]
                    yb = wk.tile([128, SBLK, 2, 14], BF16, tag="yb")
                    yt = wk.tile([128, SBLK, 2, 14], BF16, tag="yt")
                    z4 = zb[:].rearrange("p s x (y c) -> p s x y c", y=2)
                    nc.vector.tensor_tensor(
                        yb[:], z4[:, :, :, 0, :],
                        wbc(gy, 14)[:, :, None, :].broadcast_to([128, SBLK, 2, 14]),
                        OP.mult)
                    nc.vector.tensor_tensor(
                        yt[:], z4[:, :, :, 1, :],
                        wbc(fy, 14)[:, :, None, :].broadcast_to([128, SBLK, 2, 14]),
                        OP.mult)
                    nc.vector.tensor_tensor(yb[:], yb[:], yt[:], OP.add)
                    # x-blend -> k0b (ch 2..13) and denstd (ch 0..1)
                    xb = wk.tile([128, SBLK, 14], BF16, tag="xb")
                    xt = wk.tile([128, SBLK, 14], BF16, tag="xt")
                    nc.vector.tensor_tensor(
                        xb[:], yb[:, :, 0, :], wbc(gx, 14), OP.mult)
                    nc.vector.tensor_tensor(
                        xt[:], yb[:, :, 1, :], wbc(fx, 14), OP.mult)
                    nc.vector.tensor_tensor(xb[:], xb[:], xt[:], OP.add)
                    nc.vector.tensor_copy(k0b[:, sb:sb + SBLK, :], xb[:, :, 2:14])
                    nc.vector.tensor_copy(denstd[:, sb:sb + SBLK, :], xb[:, :, 0:2])

                # ---- stage E: MLP per 4-sample block ----
                for mb in range(S // MBLK):
                    sb = mb * MBLK
                    feat = mp.tile([128, MBLK, 39], F32, tag="feat")
                    nc.vector.tensor_copy(feat[:, :, 0:12], k0b[:, sb:sb + MBLK, :])
                    nc.vector.tensor_copy(
                        feat[:, :, 12:39],
                        vemb[:, None, :].broadcast_to([128, MBLK, 27]))
                    ftp = psp.tile([39, 128 * MBLK], F32, tag="ftp")
                    for si in range(MBLK):
                        nc.tensor.transpose(
                            out=ftp[:, si * 128:(si + 1) * 128],
                            in_=feat[:, si, :], identity=idt[:])
                    fts = mp.tile([39, 128 * MBLK], F32, tag="fts")
                    nc.scalar.activation(fts[:], ftp[:], AF.Copy)
                    h0p = ps1.tile([128, 128 * MBLK], F32, tag="h0p")
                    nc.tensor.matmul(out=h0p[:], lhsT=w0t[:], rhs=fts[:],
                                     start=True, stop=True)
                    h0 = mp.tile([128, 128 * MBLK], F32, tag="h0")
                    nc.scalar.activation(h0[:], h0p[:], AF.Relu, bias=b0t[:])
                    h1p = ps1.tile([128, 128 * MBLK], F32, tag="h1p")
                    nc.tensor.matmul(out=h1p[:], lhsT=w1t[:], rhs=h0[:],
                                     start=True, stop=True)
                    h1 = mp.tile([128, 128 * MBLK], F32, tag="h1")
                    nc.scalar.activation(h1[:], h1p[:], AF.Relu, bias=b1t[:])
                    o6p = ps2.tile([6, 128 * MBLK], F32, tag="o6p")
                    nc.tensor.matmul(out=o6p[:], lhsT=w2t[:], rhs=h1[:],
                                     start=True, stop=True)
                    o6 = mp.tile([6, 128 * MBLK], F32, tag="o6")
                    nc.vector.tensor_scalar(o6[:], o6p[:], b2t[:], None, OP.add)
                    obp_ = ps2.tile([128, MBLK * 6], F32, tag="obp")
                    for si in range(MBLK):
                        nc.tensor.transpose(
                            out=obp_[:, si * 6:(si + 1) * 6],
                            in_=o6[:, si * 128:(si + 1) * 128],
                            identity=idt[0:6, 0:6])
                    nc.scalar.activation(
                        m6[:, sb:sb + MBLK, :].rearrange("p s c -> p (s c)"),
                        obp_[:], AF.Copy)

                # ---- stage D: weights pipeline [128, K*S] ----
                stdsp = wk.tile([128, S], F32, tag="stdsp")
                nc.scalar.activation(stdsp[:], denstd[:, :, 1], AF.Exp)
                nc.scalar.activation(stdsp[:], stdsp[:], AF.Ln, bias=1.0)
                dk = bp.tile([128, K, S], F32, tag="dk")
                nc.vector.tensor_tensor(
                    dk[:], stdsp[:, None, :].broadcast_to([128, K, S]),
                    ep[:, :, None].broadcast_to([128, K, S]), OP.mult)
                nc.vector.tensor_tensor(
                    dk[:], dk[:],
                    denstd[:, None, :, 0].broadcast_to([128, K, S]), OP.add)
                nc.scalar.activation(dk[:], dk[:], AF.Exp, bias=ACT_SHIFT)
                nc.scalar.activation(dk[:], dk[:], AF.Ln, bias=1.0)
                nc.scalar.activation(dk[:], dk[:], AF.Exp, scale=-INTERVAL)  # p
                tin = bp.tile([128, K, S], F32, tag="tin")
                for k in range(K):
                    nc.vector.tensor_tensor_scan(
                        tin[:, k, :], dk[:, k, :], dk[:, k, :], 1.0,
                        OP.mult, OP.bypass)
                wgt = bp.tile([128, K, S], F32, tag="wgt")
                # w[:, k, 1:] = Tin[:, k, :-1] - Tin[:, k, 1:]; w[:, k, 0] = 1 - Tin[:, k, 0]
                tflat = tin[:].rearrange("p k s -> p (k s)")
                wflat = wgt[:].rearrange("p k s -> p (k s)")
                nc.vector.tensor_tensor(
                    wflat[:, 1:], tflat[:, 0:K * S - 1], tflat[:, 1:], OP.subtract)
                nc.vector.tensor_scalar(
                    wgt[:, :, 0], tin[:, :, 0], -1.0, 1.0, OP.mult, OP.add)
                accs = wk.tile([128, 1], F32, tag="accs")
                nc.vector.tensor_reduce(accs[:], wflat[:], mybir.AxisListType.X, OP.add)

                # ---- stage F: rgb + reduction ----
                rsp = wk.tile([128, S, 3], F32, tag="rsp")
                nc.scalar.activation(rsp[:], m6[:, :, 3:6], AF.Exp)
                nc.scalar.activation(rsp[:], rsp[:], AF.Ln, bias=1.0)
                args = bp.tile([128, K, S, 3], BF16, tag="args")
                nc.vector.tensor_tensor(
                    args[:], rsp[:, None, :, :].broadcast_to([128, K, S, 3]),
                    epr[:].rearrange("p (k c) -> p k c", c=3)[:, :, None, :].broadcast_to([128, K, S, 3]),
                    OP.mult)
                nc.vector.tensor_tensor(
                    args[:], args[:],
                    m6[:, None, :, 0:3].broadcast_to([128, K, S, 3]), OP.add)
                nc.scalar.activation(args[:], args[:], AF.Sigmoid)
                wtn = bp.tile([128, K, S, 3], BF16, tag="wtn")
                nc.vector.tensor_tensor(
                    wtn[:], args[:],
                    wgt[:, :, :, None].broadcast_to([128, K, S, 3]), OP.mult)
                st3 = wk.tile([128, 3], F32, tag="st3")
                for c in range(3):
                    nc.vector.tensor_reduce(
                        st3[:, c:c + 1],
                        wtn[:].rearrange("p k s c -> p (k s) c")[:, :, c],
                        mybir.AxisListType.X, OP.add)
                oout = wk.tile([128, 3], F32, tag="oout")
                nc.vector.tensor_tensor(
                    st3[:], st3[:], accs[:].broadcast_to([128, 3]), OP.subtract)
                nc.vector.tensor_scalar(oout[:], st3[:], 1.0 / K, 1.0, OP.mult, OP.add)
                nc.sync.dma_start(out=out[r0:r0 + 128, :], in_=oout[:])
    nc.compile()
    return nc


_PROG = None
_GRIDP_CACHE = {}


def _repack(density, density_std, k0):
    key = id(density)
    if key in _GRIDP_CACHE:
        return _GRIDP_CACHE[key]
    ch = np.concatenate([np.asarray(density), np.asarray(density_std),
                         np.asarray(k0)], axis=0)       # [14,X,Y,Z]
    ch = np.moveaxis(ch, 0, -1)                          # [X,Y,Z,14]
    out = np.zeros((G, G, G, 28), dtype=bfm)
    out[:, :, :, :14] = ch.astype(bfm)
    out[:, :-1, :, 14:] = ch[:, 1:, :, :].astype(bfm)
    out = np.ascontiguousarray(out.reshape(NENT, 28))
    _GRIDP_CACHE.clear()
    _GRIDP_CACHE[key] = out
    return out


def kernel(rays_o, rays_d, density_grid, density_std_grid, k0_grid,
           w0, b0, w1, b1, w2, b2, eps_den, eps_rgb):
    global _PROG
    import os
    if _PROG is None:
        _PROG = build_program()
    gridp = _repack(density_grid, density_std_grid, k0_grid)
    t = (NEAR + STEPDIST * np.arange(S, dtype=np.float32))
    t_rep = np.tile(t[None, :], (128, 1))
    freq = (2.0 ** np.arange(4, dtype=np.float32))
    freq_rep = np.tile(np.tile(freq, 3)[None, :], (128, 1))
    eps_rep = np.tile(np.asarray(eps_den, np.float32)[None, :], (128, 1))
    epsr_rep = np.tile(np.asarray(eps_rgb, np.float32).reshape(-1)[None, :], (128, 1))
    ident = np.eye(128, dtype=np.float32)
    shared = dict(
        gridp=gridp, t_rep=t_rep, freq_rep=freq_rep, eps_rep=eps_rep,
        epsr_rep=epsr_rep,
        w0T=np.asarray(w0, np.float32),
        w1T=np.asarray(w1, np.float32),
        w2T=np.asarray(w2, np.float32),
        b0c=np.asarray(b0, np.float32).reshape(128, 1),
        b1c=np.asarray(b1, np.float32).reshape(128, 1),
        b2c=np.asarray(b2, np.float32).reshape(6, 1),
        identd=ident)
    rays_o = np.asarray(rays_o, np.float32)
    rays_d = np.asarray(rays_d, np.float32)
    in_maps = []
    for c in range(NC):
        m = dict(shared)
        m["rays_o"] = rays_o[c * RPC:(c + 1) * RPC]
        m["rays_d"] = rays_d[c * RPC:(c + 1) * RPC]
        rdc = m["rays_d"]
        vdc = rdc / np.linalg.norm(rdc, axis=-1, keepdims=True)
        angc = vdc[:, :, None] * (2.0 ** np.arange(4, dtype=np.float32))[None, None, :]
        m["vembd"] = np.ascontiguousarray(np.concatenate(
            [vdc, np.sin(angc).reshape(-1, 12), np.cos(angc).reshape(-1, 12)],
            axis=-1).astype(np.float32))
        in_maps.append(m)
    trace = bool(int(os.environ.get("KERNEL_TRACE", "0")))
    if trace:
        import ntff_hook
        ntff_hook.install_ntff_hook()
    res = run_bass_kernel_spmd(_PROG, in_maps, core_ids=list(range(NC)),
                               trace=trace)
    if trace and res.exec_time_ns is not None:
        print(f"HW exec time: {res.exec_time_ns} ns")
    return np.concatenate([r["out"] for r in res.results], axis=0)



# revision 11
# speedup vs baseline: 1.6489x; 1.6489x over previous
"""Trainium2 Bass kernel for nn_DirectContractedVoxGO_Sto.

Data-parallel over rays: 8 cores x 512 rays. Grids repacked host-side to
[x,y,z,56] bf16 (14 channels at the four (x,y) corners) so ONE contiguous
112-value window (rows z and z+1) covers the whole 2x2x2 trilinear cube;
one batched indirect-DMA per 32-sample block (4096 descriptors of 224B).
On-chip: separable trilinear blend (DVE), transmittance via
tensor_tensor_scan cumprod, tiny MLP on PE with per-sample transposes,
stochastic RGB via sigmoid, and a fused (s,k) reduction.
"""
import numpy as np
import ml_dtypes

import concourse.bass as bass
import concourse.bacc as bacc
import concourse.mybir as mybir
import concourse.tile as tile
from concourse.bass_utils import run_bass_kernel_spmd

bfm = ml_dtypes.bfloat16
F32 = mybir.dt.float32
BF16 = mybir.dt.bfloat16
I32 = mybir.dt.int32
AF = mybir.ActivationFunctionType
OP = mybir.AluOpType

# problem constants (hardcoded; kernel.py must be self-contained)
G = 160
S = 256
K = 12
RPC = 512            # rays per core
NC = 8
XYZ_MIN = -1.2
XYZ_MAX = 1.2
ACT_SHIFT = float(np.log(1.0 / (1.0 - 1e-4) - 1.0))
STEPDIST = 0.5 * (XYZ_MAX - XYZ_MIN) / G
NEAR = 0.1
INTERVAL = 0.5
NENT = G * G * G
USCL = (G - 1) / (XYZ_MAX - XYZ_MIN)
SBLK = 32            # samples per blend block
MBLK = 4             # samples per MLP block


def build_program():
    nc = bacc.Bacc("TRN2", target_bir_lowering=False, debug=False, num_devices=NC)
    for val in {float(np.pi / 2), -0.5, -1.0, ACT_SHIFT, 0.5, -float(INTERVAL)}:
        t = nc.alloc_sbuf_tensor(f"constx-{val}", [128, 1], F32)
        nc.gpsimd.memset(t.ap(), val)
        nc.const_aps.aps[(F32, val)] = t.ap()
    nc.all_engine_barrier()
    gridp = nc.dram_tensor("gridp", [NENT, 112], BF16, kind="ExternalInput")
    rays_o = nc.dram_tensor("rays_o", [RPC, 3], F32, kind="ExternalInput")
    rays_d = nc.dram_tensor("rays_d", [RPC, 3], F32, kind="ExternalInput")
    t_rep = nc.dram_tensor("t_rep", [128, S], F32, kind="ExternalInput")
    freq_rep = nc.dram_tensor("freq_rep", [128, 12], F32, kind="ExternalInput")
    eps_rep = nc.dram_tensor("eps_rep", [128, K], F32, kind="ExternalInput")
    epsr_rep = nc.dram_tensor("epsr_rep", [128, K * 3], BF16, kind="ExternalInput")
    w0T = nc.dram_tensor("w0T", [39, 128], BF16, kind="ExternalInput")
    w1T = nc.dram_tensor("w1T", [128, 128], BF16, kind="ExternalInput")
    w2T = nc.dram_tensor("w2T", [128, 6], BF16, kind="ExternalInput")
    b0c = nc.dram_tensor("b0c", [128, 1], F32, kind="ExternalInput")
    b1c = nc.dram_tensor("b1c", [128, 1], F32, kind="ExternalInput")
    b2c = nc.dram_tensor("b2c", [6, 1], F32, kind="ExternalInput")
    identd = nc.dram_tensor("identd", [128, 128], F32, kind="ExternalInput")
    vembd = nc.dram_tensor("vembd", [RPC, 27], F32, kind="ExternalInput")
    out = nc.dram_tensor("out", [RPC, 3], F32, kind="ExternalOutput")

    with tile.TileContext(nc) as tc:
        with tc.tile_pool(name="const", bufs=1) as cp, \
             tc.tile_pool(name="big", bufs=1) as bp, \
             tc.tile_pool(name="wk", bufs=2) as wk, \
             tc.tile_pool(name="win", bufs=2) as winp, \
             tc.tile_pool(name="mlp", bufs=1) as mp, \
             tc.tile_pool(name="ps", bufs=2, space="PSUM") as psp, \
             tc.tile_pool(name="ps1", bufs=1, space="PSUM") as ps1, \
             tc.tile_pool(name="ps2", bufs=1, space="PSUM") as ps2:

            tt = cp.tile([128, S], F32, tag="t_rep")
            nc.sync.dma_start(out=tt[:], in_=t_rep[:])
            fq = cp.tile([128, 12], F32, tag="freq")
            nc.sync.dma_start(out=fq[:], in_=freq_rep[:])
            ep = cp.tile([128, K], F32, tag="eps")
            nc.sync.dma_start(out=ep[:], in_=eps_rep[:])
            epr = cp.tile([128, K * 3], BF16, tag="epsr")
            nc.sync.dma_start(out=epr[:], in_=epsr_rep[:])
            w0t = cp.tile([39, 128], BF16, tag="w0")
            nc.sync.dma_start(out=w0t[:], in_=w0T[:])
            w1t = cp.tile([128, 128], BF16, tag="w1")
            nc.sync.dma_start(out=w1t[:], in_=w1T[:])
            w2t = cp.tile([128, 6], BF16, tag="w2")
            nc.sync.dma_start(out=w2t[:], in_=w2T[:])
            b0t = cp.tile([128, 1], F32, tag="b0")
            nc.sync.dma_start(out=b0t[:], in_=b0c[:])
            b1t = cp.tile([128, 1], F32, tag="b1")
            nc.sync.dma_start(out=b1t[:], in_=b1c[:])
            b2t = cp.tile([6, 1], F32, tag="b2")
            nc.sync.dma_start(out=b2t[:], in_=b2c[:])
            idt = cp.tile([128, 128], F32, tag="ident")
            nc.sync.dma_start(out=idt[:], in_=identd[:])
            idtb = cp.tile([128, 128], BF16, tag="identb")
            nc.vector.tensor_copy(idtb[:], idt[:])

            for rt in range(RPC // 128):
                r0 = rt * 128
                # ---- stage 0: ray setup ----
                ro = wk.tile([128, 3], F32, tag="ro")
                nc.sync.dma_start(out=ro[:], in_=rays_o[r0:r0 + 128, :])
                rd = wk.tile([128, 3], F32, tag="rd")
                nc.sync.dma_start(out=rd[:], in_=rays_d[r0:r0 + 128, :])
                rdsq = wk.tile([128, 3], F32, tag="rdsq")
                nc.scalar.activation(rdsq[:], rd[:], AF.Square)
                n2 = wk.tile([128, 1], F32, tag="n2")
                nc.vector.tensor_reduce(n2[:], rdsq[:], mybir.AxisListType.X, OP.add)
                nc.scalar.activation(n2[:], n2[:], AF.Ln)
                nc.scalar.activation(n2[:], n2[:], AF.Exp, scale=-0.5)  # rsqrt
                vd = wk.tile([128, 3], F32, tag="vd")
                nc.vector.tensor_scalar(vd[:], rd[:], n2[:], None, OP.mult)
                # vemb [128, 27] = [vd, sin, cos]  (host-computed)
                vemb = wk.tile([128, 27], F32, tag="vemb")
                nc.sync.dma_start(out=vemb[:], in_=vembd[r0:r0 + 128, :])
                # ---- stage A: pts / contraction / indices [128, S] ----
                pc = []
                for c in range(3):
                    pct = wk.tile([128, S], F32, tag=f"p{c}", name=f"p{c}")
                    pc.append(pct)
                for c in range(3):
                    nc.vector.tensor_scalar(pc[c][:], tt[:], vd[:, c:c + 1],
                                            ro[:, c:c + 1], OP.mult, OP.add)
                m = wk.tile([128, S], F32, tag="m")
                ab = wk.tile([128, S], F32, tag="ab")
                nc.scalar.activation(m[:], pc[0][:], AF.Abs)
                nc.scalar.activation(ab[:], pc[1][:], AF.Abs)
                nc.vector.tensor_tensor(m[:], m[:], ab[:], OP.max)
                nc.scalar.activation(ab[:], pc[2][:], AF.Abs)
                nc.vector.tensor_tensor(m[:], m[:], ab[:], OP.max)
                nc.vector.tensor_scalar(m[:], m[:], 1.0, None, OP.max)
                nc.scalar.activation(m[:], m[:], AF.Ln)
                rr = wk.tile([128, S], F32, tag="rr")
                nc.scalar.activation(rr[:], m[:], AF.Exp, scale=-1.0)
                rsq = wk.tile([128, S], F32, tag="rsq")
                nc.scalar.activation(rsq[:], rr[:], AF.Square)
                ff = wk.tile([128, S], F32, tag="ff")
                nc.vector.tensor_scalar(ff[:], rr[:], 1.2, None, OP.mult)
                nc.vector.tensor_scalar(rsq[:], rsq[:], 0.2, None, OP.mult)
                nc.vector.tensor_tensor(ff[:], ff[:], rsq[:], OP.subtract)
                frs = []
                ent = wk.tile([128, S], F32, tag="ent")
                for c in range(3):
                    u = wk.tile([128, S], F32, tag=f"u{c}")
                    nc.vector.tensor_tensor(u[:], pc[c][:], ff[:], OP.mult)
                    nc.vector.tensor_scalar(u[:], u[:], USCL, -XYZ_MIN * USCL,
                                            OP.mult, OP.add)
                    nc.vector.tensor_scalar(u[:], u[:], 0.0, None, OP.max)
                    fm = wk.tile([128, S], I32, tag=f"fm{c}", name="fm")
                    nc.vector.tensor_copy(fm[:], u[:])
                    i0 = wk.tile([128, S], F32, tag=f"i0{c}", name="i0")
                    nc.vector.tensor_copy(i0[:], fm[:])
                    gtm = wk.tile([128, S], F32, tag=f"gtm{c}", name="gtm")
                    nc.vector.tensor_tensor(gtm[:], i0[:], u[:], OP.is_gt)
                    nc.vector.tensor_tensor(i0[:], i0[:], gtm[:], OP.subtract)
                    nc.vector.tensor_scalar(i0[:], i0[:], float(G - 2), None, OP.min)
                    fr = wk.tile([128, S], F32, tag=f"fr{c}")
                    nc.vector.tensor_tensor(fr[:], u[:], i0[:], OP.subtract)
                    frs.append(fr)
                    if c == 0:
                        nc.vector.tensor_scalar(ent[:], i0[:], float(G), None, OP.mult)
                    elif c == 1:
                        nc.vector.tensor_tensor(ent[:], ent[:], i0[:], OP.add)
                        nc.vector.tensor_scalar(ent[:], ent[:], float(G), None, OP.mult)
                    else:
                        nc.vector.tensor_tensor(ent[:], ent[:], i0[:], OP.add)
                gxs = []
                for c in range(3):
                    gg = wk.tile([128, S], F32, tag=f"g{c}")
                    nc.vector.tensor_scalar(gg[:], frs[c][:], -1.0, 1.0, OP.mult, OP.add)
                    gxs.append(gg)
                fx, fy, fz = frs
                gx, gy, gz = gxs
                e0i = wk.tile([128, S], I32, tag="e0i")
                nc.vector.tensor_copy(e0i[:], ent[:])

                # big per-rtile tiles
                k0b = bp.tile([128, S, 12], BF16, tag="k0b")
                denstd = bp.tile([128, S, 2], F32, tag="denstd")
                m6 = bp.tile([128, S, 6], BF16, tag="m6")

                # ---- stages B+C: gather + blend, per 32-sample block ----
                # win layout per sample: [dz=2][dy=2][dx=2][14ch]
                for blk in range(S // SBLK):
                    sb = blk * SBLK
                    win = winp.tile([128, SBLK, 112], BF16, tag="win")
                    for si in range(SBLK):
                        nc.gpsimd.indirect_dma_start(
                            out=win[:, si, :], out_offset=None, in_=gridp[:],
                            in_offset=bass.IndirectOffsetOnAxis(
                                ap=e0i[:, sb + si:sb + si + 1], axis=0))
                    # blend weights as [128, SBLK] slices, broadcast over chans
                    def wbc(t, n):
                        return t[:, sb:sb + SBLK][:, :, None].broadcast_to(
                            [128, SBLK, n])
                    # z-blend: win [z2, 56] -> zb [128, SBLK, 56]
                    zb = wk.tile([128, SBLK, 56], BF16, tag="zb")
                    zt = wk.tile([128, SBLK, 56], BF16, tag="zt")
                    w4 = win[:].rearrange("p s (z q) -> p s z q", z=2)
                    nc.vector.tensor_tensor(
                        zb[:], w4[:, :, 0, :], wbc(gz, 56), OP.mult)
                    nc.vector.tensor_tensor(
                        zt[:], w4[:, :, 1, :], wbc(fz, 56), OP.mult)
                    nc.vector.tensor_tensor(zb[:], zb[:], zt[:], OP.add)
                    # y-blend: zb [y2, 28] -> yb [128, SBLK, 28]
                    yb = wk.tile([128, SBLK, 28], BF16, tag="yb")
                    yt = wk.tile([128, SBLK, 28], BF16, tag="yt")
                    z4 = zb[:].rearrange("p s (y c) -> p s y c", y=2)
                    nc.vector.tensor_tensor(
                        yb[:], z4[:, :, 0, :], wbc(gy, 28), OP.mult)
                    nc.vector.tensor_tensor(
                        yt[:], z4[:, :, 1, :], wbc(fy, 28), OP.mult)
                    nc.vector.tensor_tensor(yb[:], yb[:], yt[:], OP.add)
                    # x-blend -> k0b (ch 2..13) and denstd (ch 0..1)
                    xb = wk.tile([128, SBLK, 14], BF16, tag="xb")
                    xt = wk.tile([128, SBLK, 14], BF16, tag="xt")
                    y4 = yb[:].rearrange("p s (x c) -> p s x c", x=2)
                    nc.vector.tensor_tensor(
                        xb[:], y4[:, :, 0, :], wbc(gx, 14), OP.mult)
                    nc.vector.tensor_tensor(
                        xt[:], y4[:, :, 1, :], wbc(fx, 14), OP.mult)
                    nc.vector.tensor_tensor(xb[:], xb[:], xt[:], OP.add)
                    nc.vector.tensor_copy(k0b[:, sb:sb + SBLK, :], xb[:, :, 2:14])
                    nc.vector.tensor_copy(denstd[:, sb:sb + SBLK, :], xb[:, :, 0:2])

                # ---- stage E: MLP per 4-sample block ----
                for mb in range(S // MBLK):
                    sb = mb * MBLK
                    feat = mp.tile([128, MBLK, 39], BF16, tag="feat")
                    nc.vector.tensor_copy(feat[:, :, 0:12], k0b[:, sb:sb + MBLK, :])
                    nc.vector.tensor_copy(
                        feat[:, :, 12:39],
                        vemb[:, None, :].broadcast_to([128, MBLK, 27]))
                    ftp = psp.tile([39, 128 * MBLK], BF16, tag="ftp")
                    for si in range(MBLK):
                        nc.tensor.transpose(
                            out=ftp[:, si * 128:(si + 1) * 128],
                            in_=feat[:, si, :], identity=idtb[:])
                    fts = mp.tile([39, 128 * MBLK], BF16, tag="fts")
                    nc.scalar.activation(fts[:], ftp[:], AF.Copy)
                    h0p = ps1.tile([128, 128 * MBLK], F32, tag="h0p")
                    nc.tensor.matmul(out=h0p[:], lhsT=w0t[:], rhs=fts[:],
                                     start=True, stop=True)
                    h0 = mp.tile([128, 128 * MBLK], BF16, tag="h0")
                    nc.scalar.activation(h0[:], h0p[:], AF.Relu, bias=b0t[:])
                    h1p = ps1.tile([128, 128 * MBLK], F32, tag="h1p")
                    nc.tensor.matmul(out=h1p[:], lhsT=w1t[:], rhs=h0[:],
                                     start=True, stop=True)
                    h1 = mp.tile([128, 128 * MBLK], BF16, tag="h1")
                    nc.scalar.activation(h1[:], h1p[:], AF.Relu, bias=b1t[:])
                    o6p = ps2.tile([6, 128 * MBLK], F32, tag="o6p")
                    nc.tensor.matmul(out=o6p[:], lhsT=w2t[:], rhs=h1[:],
                                     start=True, stop=True)
                    o6 = mp.tile([6, 128 * MBLK], BF16, tag="o6")
                    nc.vector.tensor_scalar(o6[:], o6p[:], b2t[:], None, OP.add)
                    obp_ = ps2.tile([128, MBLK * 6], BF16, tag="obp")
                    for si in range(MBLK):
                        nc.tensor.transpose(
                            out=obp_[:, si * 6:(si + 1) * 6],
                            in_=o6[:, si * 128:(si + 1) * 128],
                            identity=idtb[0:6, 0:6])
                    nc.scalar.activation(
                        m6[:, sb:sb + MBLK, :].rearrange("p s c -> p (s c)"),
                        obp_[:], AF.Copy)

                # ---- stage D: weights pipeline [128, K*S] ----
                stdsp = wk.tile([128, S], F32, tag="stdsp")
                nc.scalar.activation(stdsp[:], denstd[:, :, 1], AF.Exp)
                nc.scalar.activation(stdsp[:], stdsp[:], AF.Ln, bias=1.0)
                dk = bp.tile([128, K, S], F32, tag="dk")
                nc.vector.tensor_tensor(
                    dk[:], stdsp[:, None, :].broadcast_to([128, K, S]),
                    ep[:, :, None].broadcast_to([128, K, S]), OP.mult)
                nc.vector.tensor_tensor(
                    dk[:], dk[:],
                    denstd[:, None, :, 0].broadcast_to([128, K, S]), OP.add)
                nc.scalar.activation(dk[:], dk[:], AF.Exp, bias=ACT_SHIFT)
                nc.scalar.activation(dk[:], dk[:], AF.Ln, bias=1.0)
                nc.scalar.activation(dk[:], dk[:], AF.Exp, scale=-INTERVAL)  # p
                tin = bp.tile([128, K, S], F32, tag="tin")
                for k in range(K):
                    nc.vector.tensor_tensor_scan(
                        tin[:, k, :], dk[:, k, :], dk[:, k, :], 1.0,
                        OP.mult, OP.bypass)
                wgt = bp.tile([128, K, S], F32, tag="wgt")
                wgtb = bp.tile([128, K, S], BF16, tag="wgtb")
                # w[:, k, 1:] = Tin[:, k, :-1] - Tin[:, k, 1:]; w[:, k, 0] = 1 - Tin[:, k, 0]
                tflat = tin[:].rearrange("p k s -> p (k s)")
                wflat = wgt[:].rearrange("p k s -> p (k s)")
                nc.vector.tensor_tensor(
                    wflat[:, 1:], tflat[:, 0:K * S - 1], tflat[:, 1:], OP.subtract)
                nc.vector.tensor_scalar(
                    wgt[:, :, 0], tin[:, :, 0], -1.0, 1.0, OP.mult, OP.add)
                accs = wk.tile([128, 1], F32, tag="accs")
                nc.vector.tensor_reduce(accs[:], wflat[:], mybir.AxisListType.X, OP.add)

                # ---- stage F: rgb + reduction ----
                rsp = wk.tile([128, S, 3], BF16, tag="rsp")
                nc.scalar.activation(rsp[:], m6[:, :, 3:6], AF.Exp)
                nc.scalar.activation(rsp[:], rsp[:], AF.Ln, bias=1.0)
                args = bp.tile([128, K, S, 3], BF16, tag="args")
                nc.vector.tensor_tensor(
                    args[:], rsp[:, None, :, :].broadcast_to([128, K, S, 3]),
                    epr[:].rearrange("p (k c) -> p k c", c=3)[:, :, None, :].broadcast_to([128, K, S, 3]),
                    OP.mult)
                nc.vector.tensor_tensor(
                    args[:], args[:],
                    m6[:, None, :, 0:3].broadcast_to([128, K, S, 3]), OP.add)
                nc.scalar.activation(args[:], args[:], AF.Sigmoid)
                wtn = bp.tile([128, K, S, 3], BF16, tag="wtn")
                nc.vector.tensor_copy(wgtb[:], wgt[:])
                nc.vector.tensor_tensor(
                    wtn[:], args[:],
                    wgtb[:, :, :, None].broadcast_to([128, K, S, 3]), OP.mult)
                st3 = wk.tile([128, 3], F32, tag="st3")
                for c in range(3):
                    nc.vector.tensor_reduce(
                        st3[:, c:c + 1],
                        wtn[:].rearrange("p k s c -> p (k s) c")[:, :, c],
                        mybir.AxisListType.X, OP.add)
                oout = wk.tile([128, 3], F32, tag="oout")
                nc.vector.tensor_tensor(
                    st3[:], st3[:], accs[:].broadcast_to([128, 3]), OP.subtract)
                nc.vector.tensor_scalar(oout[:], st3[:], 1.0 / K, 1.0, OP.mult, OP.add)
                nc.sync.dma_start(out=out[r0:r0 + 128, :], in_=oout[:])
    nc.compile()
    return nc


_PROG = None
_GRIDP_CACHE = {}


def _repack(density, density_std, k0):
    key = id(density)
    if key in _GRIDP_CACHE:
        return _GRIDP_CACHE[key]
    ch = np.concatenate([np.asarray(density), np.asarray(density_std),
                         np.asarray(k0)], axis=0)       # [14,X,Y,Z]
    ch = np.moveaxis(ch, 0, -1).astype(bfm)              # [X,Y,Z,14]
    half = np.zeros((G, G, G, 56), dtype=bfm)
    half[:, :, :, 0:14] = ch
    half[:-1, :, :, 14:28] = ch[1:]                      # dx=1
    half[:, :-1, :, 28:42] = ch[:, 1:]                   # dy=1
    half[:-1, :-1, :, 42:56] = ch[1:, 1:]                # dx=dy=1
    half = half.reshape(NENT, 56)
    out = np.zeros((NENT, 112), dtype=bfm)
    out[:, 0:56] = half                                  # z
    out[:-1, 56:112] = half[1:]                          # z+1
    out = np.ascontiguousarray(out)
    _GRIDP_CACHE.clear()
    _GRIDP_CACHE[key] = out
    return out


def kernel(rays_o, rays_d, density_grid, density_std_grid, k0_grid,
           w0, b0, w1, b1, w2, b2, eps_den, eps_rgb):
    global _PROG
    import os
    if _PROG is None:
        _PROG = build_program()
    gridp = _repack(density_grid, density_std_grid, k0_grid)
    t = (NEAR + STEPDIST * np.arange(S, dtype=np.float32))
    t_rep = np.tile(t[None, :], (128, 1))
    freq = (2.0 ** np.arange(4, dtype=np.float32))
    freq_rep = np.tile(np.tile(freq, 3)[None, :], (128, 1))
    eps_rep = np.tile(np.asarray(eps_den, np.float32)[None, :], (128, 1))
    epsr_rep = np.tile(np.asarray(eps_rgb, np.float32).reshape(-1)[None, :],
                       (128, 1)).astype(bfm)
    ident = np.eye(128, dtype=np.float32)
    shared = dict(
        gridp=gridp, t_rep=t_rep, freq_rep=freq_rep, eps_rep=eps_rep,
        epsr_rep=epsr_rep,
        w0T=np.asarray(w0, np.float32).astype(bfm),
        w1T=np.asarray(w1, np.float32).astype(bfm),
        w2T=np.asarray(w2, np.float32).astype(bfm),
        b0c=np.asarray(b0, np.float32).reshape(128, 1),
        b1c=np.asarray(b1, np.float32).reshape(128, 1),
        b2c=np.asarray(b2, np.float32).reshape(6, 1),
        identd=ident)
    rays_o = np.asarray(rays_o, np.float32)
    rays_d = np.asarray(rays_d, np.float32)
    in_maps = []
    for c in range(NC):
        m = dict(shared)
        m["rays_o"] = rays_o[c * RPC:(c + 1) * RPC]
        m["rays_d"] = rays_d[c * RPC:(c + 1) * RPC]
        rdc = m["rays_d"]
        vdc = rdc / np.linalg.norm(rdc, axis=-1, keepdims=True)
        angc = vdc[:, :, None] * (2.0 ** np.arange(4, dtype=np.float32))[None, None, :]
        m["vembd"] = np.ascontiguousarray(np.concatenate(
            [vdc, np.sin(angc).reshape(-1, 12), np.cos(angc).reshape(-1, 12)],
            axis=-1).astype(np.float32))
        in_maps.append(m)
    trace = bool(int(os.environ.get("KERNEL_TRACE", "0")))
    if trace:
        import ntff_hook
        ntff_hook.install_ntff_hook()
    res = run_bass_kernel_spmd(_PROG, in_maps, core_ids=list(range(NC)),
                               trace=trace)
    if trace and res.exec_time_ns is not None:
        print(f"HW exec time: {res.exec_time_ns} ns")
    return np.concatenate([r["out"] for r in res.results], axis=0)

